# revision 1
# baseline (speedup 1.0000x reference)
"""
AM-Softmax + intra-class loss kernel for Trainium2, 8 NeuronCores.

Strategy (class-sharded distributed softmax, fp8 DoubleRow matmul):
  * Classes C=20000 sharded 2500/core (padded 2560). Host normalizes E and W
    rows to unit norm (the per-row scale that fp8 quantization needs anyway),
    scales by 16 and casts to fp8-e4m3. The AM scale (30) and the two 1/16
    factors ride the exp's constant scale, so no norm work runs on device.
  * Z tiles come from fp8 DoubleRow matmuls: one instruction contracts the
    full K=256 (two 128-deep k-tiles) per 512-wide PSUM bank -- 4x the f32r
    rate on the PE.
  * Per 128-row chunk (32 chunks): ACT exps cols [0:1536] with the fused
    per-partition accumulator (exact exp, logsumexp offset -30); DVE covers
    cols [1536:2500] with a Schraudolph exp built for bf16: i16 = round(
    a*z + b) is bf16's bit pattern of ~exp(z-30), then one
    scalar_tensor_tensor folds the two halves and row-sums in a single
    pass. The ~1.5e-3 systematic error of the piecewise-linear exp sits far
    under the tolerance; the label-logit correction uses exact f32 dots.
  * Label logits: host gathers normalized W[labels]; device does 4 fused
    dot-product instructions (scalar_tensor_tensor accumulate).
  * Intra-class term: group-sum trick on normalized eg rows (bf16
    sel-matmul), sum((1 - e_i.e_j)) = 28 - (||sum_g e||^2 - 8)/2 per group.
  * Host combine is O(B) float64.
"""

import numpy as np
import ml_dtypes

import concourse.bacc as bacc
import concourse.tile as tile
from concourse import mybir
from concourse.bass_utils import run_bass_kernel_spmd
from contextlib import ExitStack

B = 4096
D = 256
C = 20000
G = 512
NSAMP = 8
NCORES = 8
CREAL = C // NCORES          # 2500 real classes per core
CSH = 2560                   # padded classes per core
RCH = B // 128               # 32 row chunks
RPC = B // NCORES            # 512 rows per core (label-cos shard)
GPC = G // NCORES            # 64 groups per core
NA = 1476                    # ACT cols per chunk (exact exp)
NB = CREAL - NA              # 964 DVE cols per chunk (Schraudolph)

AM_MARGIN = 0.3
AM_SCALE = 30.0
INTRA_MARGIN = 0.5
LAMBDA_INTRA = 0.1
OFF = 30.0                   # logsumexp offset
QS = 16.0                    # fp8 quantization scale on each operand
ZSCALE = AM_SCALE / (QS * QS)   # psum -> s*cos

# bf16-space Schraudolph: i16 = round(z*A16 + B16) is bf16 bits of ~exp(z)
A16 = float(2**7 / np.log(2))
B16 = float(127 * 2**7 - 0.927)
SCH_MUL = A16 * ZSCALE
SCH_ADD = B16 - A16 * OFF

F32 = mybir.dt.float32
BF16 = mybir.dt.bfloat16
F8 = mybir.dt.float8e4
I16 = mybir.dt.int16
AF = mybir.ActivationFunctionType
ALU = mybir.AluOpType
AXL = mybir.AxisListType
PM = mybir.MatmulPerfMode


def build_program():
    nc = bacc.Bacc("TRN2", target_bir_lowering=False)

    etq_d = nc.dram_tensor("etq", [128, 2, B], F8, kind="ExternalInput")
    wtq_d = nc.dram_tensor("wtq", [128, 2, CSH], F8, kind="ExternalInput")
    ern_d = nc.dram_tensor("ern", [128, 4, D], F32, kind="ExternalInput")
    wln_d = nc.dram_tensor("wln", [128, 4, D], F32, kind="ExternalInput")
    egn_d = nc.dram_tensor("egn", [128, 4, D], BF16, kind="ExternalInput")
    sel_d = nc.dram_tensor("sel", [128, GPC], BF16, kind="ExternalInput")

    outa_d = nc.dram_tensor("out_a", [128, RCH], F32, kind="ExternalOutput")
    outb_d = nc.dram_tensor("out_b", [128, RCH], F32, kind="ExternalOutput")
    outlc_d = nc.dram_tensor("out_lc", [128, 4], F32, kind="ExternalOutput")
    outiv_d = nc.dram_tensor("out_iv", [GPC, 1], F32, kind="ExternalOutput")

    with tile.TileContext(nc) as tc, ExitStack() as ctx:
        big = ctx.enter_context(tc.tile_pool(name="big", bufs=1))
        scr = ctx.enter_context(tc.tile_pool(name="scr", bufs=3))
        psum = ctx.enter_context(tc.tile_pool(name="psum", bufs=2, space="PSUM"))

        ETQ = big.tile([128, 2, B], F8)
        WTQ = big.tile([128, 2, CSH], F8)
        ernsb = big.tile([128, 4, D], F32)
        wlnsb = big.tile([128, 4, D], F32)
        egnsb = big.tile([128, 4, D], BF16)
        selsb = big.tile([128, GPC], BF16)

        # DMAs: the B-columns of W gate the DVE critical path -> first on SP;
        # E.T's first quarter rides the ACT queue concurrently.
        nc.scalar.dma_start(out=ETQ[:, :, 0:1024], in_=etq_d[:][:, :, 0:1024])
        nc.sync.dma_start(out=WTQ[:, :, NA:CSH], in_=wtq_d[:][:, :, NA:CSH])
        nc.sync.dma_start(out=WTQ[:, :, 0:NA], in_=wtq_d[:][:, :, 0:NA])
        for q in range(1, 4):
            nc.sync.dma_start(out=ETQ[:, :, q * 1024:(q + 1) * 1024],
                              in_=etq_d[:][:, :, q * 1024:(q + 1) * 1024])
        nc.sync.dma_start(out=selsb, in_=sel_d[:])
        nc.sync.dma_start(out=egnsb, in_=egn_d[:])
        nc.sync.dma_start(out=ernsb, in_=ern_d[:])
        nc.sync.dma_start(out=wlnsb, in_=wln_d[:])

        negoff = big.tile([128, 1], F32)
        nc.vector.memset(negoff, -OFF)
        tsA = big.tile([128, RCH], F32)
        tsB = big.tile([128, RCH], F32)
        lcpack = big.tile([128, 4], F32)

        npairs = NSAMP * (NSAMP - 1) / 2.0
        ssq = big.tile([GPC, 1], F32)
        iv = big.tile([GPC, 1], F32)

        # ---------------- main loop ----------------
        # The halves-add+row-sum (stt) for chunk r is issued one chunk
        # late so it pipelines against the next chunk's B matmuls.
        from collections import deque
        pending = deque()  # (sch_tile, r) whose stt is still to be issued
        for r in range(RCH):
            if r == 3:
                # intra group-sum matmuls ride the PE while it idles early
                sg = psum.tile([GPC, D], F32, tag="mm")
                for j in range(4):
                    nc.tensor.matmul(sg, lhsT=selsb, rhs=egnsb[:, j],
                                     start=(j == 0), stop=(j == 3))
            if r == 5:
                # intra tail math rides early DVE gaps; frees the sg bank
                sgsb = scr.tile([GPC, D], F32, tag="sgsb")
                nc.vector.tensor_copy(out=sgsb, in_=sg)
                junk2 = scr.tile([GPC, D], F32, tag="sgj")
                nc.vector.scalar_tensor_tensor(
                    out=junk2, in0=sgsb, scalar=1.0, in1=sgsb,
                    op0=ALU.mult, op1=ALU.mult, accum_out=ssq)
                nc.vector.tensor_scalar(out=iv, in0=ssq,
                                        scalar1=-1.0 / (2.0 * npairs),
                                        scalar2=(1.0 - INTRA_MARGIN) + NSAMP / (2.0 * npairs),
                                        op0=ALU.mult, op1=ALU.add)
                nc.vector.tensor_scalar_max(iv, iv, 0.0)
                nc.sync.dma_start(out=outiv_d[:], in_=iv)
            lhs = ETQ[:, :, r * 128:(r + 1) * 128]
            # B range in two single-buffered 1-bank tiles with their own
            # tags: each refills as soon as its own transform finishes, and
            # the A tiles alternate slots so ACT's chain spans two chunks.
            ptB1 = psum.tile([128, 512], F32, tag="b1", bufs=1)
            nc.tensor.matmul(ptB1, lhsT=lhs,
                             rhs=WTQ[:, :, NA:NA + 512],
                             start=True, stop=True, perf_mode=PM.DoubleRow)
            ptB2 = psum.tile([128, 512], F32, tag="b2", bufs=1)
            nc.tensor.matmul(ptB2, lhsT=lhs,
                             rhs=WTQ[:, :, NA + 512:NA + 1024],
                             start=True, stop=True, perf_mode=PM.DoubleRow)
            # A tile: cols 0:1476, exact exp on ACT with fused row-accum
            ptA = psum.tile([128, NA], F32, tag="mm")
            for tb in range(3):
                c1 = min((tb + 1) * 512, NA)
                nc.tensor.matmul(ptA[:, tb * 512:c1], lhsT=lhs,
                                 rhs=WTQ[:, :, tb * 512:c1],
                                 start=True, stop=True, perf_mode=PM.DoubleRow)
            # Schraudolph transform of the 964 real cols; fused halves-add +
            # row-sum via scalar_tensor_tensor.
            if len(pending) >= 1:
                # fused row-sum: bf16 copy-with-accumulate runs in the DVE's
                # 4x perf mode (0.26 ns/elem vs 1.04 for the plain reduce)
                psch, pr = pending.popleft()
                stsc = scr.tile([128, NB], BF16, tag="stsc")
                nc.vector.tensor_scalar(
                    out=stsc, in0=psch.bitcast(BF16)[:, 0:NB], scalar1=1.0,
                    scalar2=0.0, op0=ALU.mult, op1=ALU.add,
                    accum_out=tsB[:, pr:pr + 1])
            sch = scr.tile([128, 1024], I16, tag="sch", bufs=4)
            nc.vector.tensor_scalar(out=sch[:, 0:512], in0=ptB1,
                                    scalar1=SCH_MUL, scalar2=SCH_ADD,
                                    op0=ALU.mult, op1=ALU.add)
            nc.vector.tensor_scalar(out=sch[:, 512:NB], in0=ptB2[:, 0:NB - 512],
                                    scalar1=SCH_MUL, scalar2=SCH_ADD,
                                    op0=ALU.mult, op1=ALU.add)
            pending.append((sch, r))
            sA = scr.tile([128, NA], F32, tag="expA")
            nc.scalar.activation(out=sA, in_=ptA, func=AF.Exp,
                                 scale=ZSCALE, bias=negoff[:, 0:1],
                                 accum_out=tsA[:, r:r + 1])
            if r == RCH - 8:
                # drain the filled portion of the sums early so the final
                # output DMAs only carry the last columns
                nc.sync.dma_start(out=outa_d[:][:, 0:RCH - 8], in_=tsA[:, 0:RCH - 8])
                nc.sync.dma_start(out=outb_d[:][:, 0:RCH - 8], in_=tsB[:, 0:RCH - 8])

        while pending:
            psch, pr = pending.popleft()
            stsc = scr.tile([128, NB], BF16, tag="stsc")
            nc.vector.tensor_scalar(
                out=stsc, in0=psch.bitcast(BF16)[:, 0:NB], scalar1=1.0,
                scalar2=0.0, op0=ALU.mult, op1=ALU.add,
                accum_out=tsB[:, pr:pr + 1])
        # label-cos dots in the tail: DVE runs them in the shadow of ACT's
        # final exp instructions
        for j in range(4):
            junk = scr.tile([128, D], F32, tag="lcj")
            nc.vector.scalar_tensor_tensor(
                out=junk, in0=ernsb[:, j], scalar=1.0, in1=wlnsb[:, j],
                op0=ALU.mult, op1=ALU.mult, accum_out=lcpack[:, j:j + 1])
        nc.sync.dma_start(out=outa_d[:][:, RCH - 8:RCH], in_=tsA[:, RCH - 8:RCH])
        nc.sync.dma_start(out=outb_d[:][:, RCH - 8:RCH], in_=tsB[:, RCH - 8:RCH])
        nc.sync.dma_start(out=outlc_d[:], in_=lcpack)

    nc.finalize()
    return nc


def kernel(embeddings, labels, weight):
    e = np.ascontiguousarray(embeddings, dtype=np.float32)
    lab = np.asarray(labels).astype(np.int64)
    w = np.ascontiguousarray(weight, dtype=np.float32)
    assert e.shape == (B, D) and w.shape == (C, D) and lab.shape == (B,)

    En = (e / np.linalg.norm(e, axis=1, keepdims=True)).astype(np.float32)
    Wn = (w / np.linalg.norm(w, axis=1, keepdims=True)).astype(np.float32)
    Eq = (QS * En).astype(ml_dtypes.float8_e4m3fn)
    etq = np.ascontiguousarray(
        Eq.T.reshape(2, 128, B).transpose(1, 0, 2))          # [128, 2, B]

    members = np.argsort(lab, kind="stable").reshape(G, NSAMP)
    assert np.all(lab[members[:, 0]] == np.arange(G))
    sel = np.tile(np.eye(GPC, dtype=np.float32), (2, 1)).astype(ml_dtypes.bfloat16)

    in_maps = []
    for k in range(NCORES):
        wsh = np.zeros((CSH, D), np.float32)
        wsh[:CREAL] = Wn[k * CREAL:(k + 1) * CREAL]
        Wq = (QS * wsh).astype(ml_dtypes.float8_e4m3fn)
        wtq = np.ascontiguousarray(Wq.T.reshape(2, 128, CSH).transpose(1, 0, 2))
        rows = slice(k * RPC, (k + 1) * RPC)
        ern = np.ascontiguousarray(
            En[rows].reshape(4, 128, D).transpose(1, 0, 2))
        wln = np.ascontiguousarray(
            Wn[lab[rows]].reshape(4, 128, D).transpose(1, 0, 2))
        gm = members[k * GPC:(k + 1) * GPC]
        eg_idx = gm.T.reshape(-1)
        egn = np.ascontiguousarray(
            En[eg_idx].reshape(4, 128, D).transpose(1, 0, 2)
        ).astype(ml_dtypes.bfloat16)
        in_maps.append({
            "etq": etq, "wtq": wtq, "ern": ern, "wln": wln,
            "egn": egn, "sel": sel,
        })

    nc = build_program()
    res = run_bass_kernel_spmd(nc, in_maps, core_ids=list(range(NCORES)))
    global _last_results
    _last_results = res

    # ---------------- host combine (O(B), float64) -----------------------
    S = np.zeros(B, np.float64)
    for k in range(NCORES):
        rk = res.results[k]
        S += (rk["out_a"].astype(np.float64) +
              rk["out_b"].astype(np.float64)).T.reshape(B)
    cl = np.concatenate(
        [res.results[k]["out_lc"][:, 0:4].astype(np.float64).T.reshape(RPC)
         for k in range(NCORES)])

    s, m = float(AM_SCALE), float(AM_MARGIN)
    S_adj = S - np.exp(s * cl - OFF) + np.exp(s * (cl - m) - OFF)
    am_i = (np.log(S_adj) + OFF) - s * (cl - m)
    am = am_i.mean()

    ivals = np.concatenate(
        [res.results[k]["out_iv"][:, 0] for k in range(NCORES)]
    ).astype(np.float64)
    intra = ivals.sum() / G
    total = am + LAMBDA_INTRA * intra
    return (np.float32(total), np.float32(am), np.float32(intra))



# revision 15
# speedup vs baseline: 1.1097x; 1.1097x over previous
"""
AM-Softmax + intra-class loss kernel for Trainium2, 8 NeuronCores.

Strategy (class-sharded, transposed-Z fp8 pipeline):
  * Classes C=20000 sharded 2500/core (padded to 20 blocks of 128). Host
    normalizes E and W rows, scales by 16, casts to fp8-e4m3 (the per-row
    scale fp8 needs anyway); the AM scale and quant factors ride the exp
    transforms' constants.
  * Z is computed TRANSPOSED: per (class-block, row-tile) the fp8 DoubleRow
    matmul emits [128 classes, 512 rows] PSUM tiles (lhsT=W-block,
    rhs=E-tile), two blocks per 2-bank psum tile.
  * exp: pair tiles alternate between ACT (native Exp -> fp8-e5m2 out) and
    DVE (Schraudolph: u8 = round(a*z+b) is the e5m2 bit pattern of
    ~exp(z-OFF); saturating f32->u8 clamps the underflow to +0.0).
  * The softmax row-sum is then a PE ones-matmul over partitions: per pair
    and 128-row slice, out[row, 1] accumulates into a [128, 32] psum tile
    over the 10 pairs (output free size 1 -> negligible PE time). No DVE
    fold passes, no ACT accumulator reads.
  * One [128, 32] copy + DMA returns all 4096 partial sums per core.
  * Label-cos dots and the intra-class term are O(B*D) epilogue work and run
    on host in float64 (the host already does the O((B+C)*D) normalize /
    quantize prologue).
"""

import numpy as np
import ml_dtypes

import concourse.bacc as bacc
import concourse.tile as tile
from concourse import mybir
from concourse.bass_utils import run_bass_kernel_spmd
from contextlib import ExitStack

B = 4096
D = 256
C = 20000
G = 512
NSAMP = 8
NCORES = 8
CREAL = C // NCORES          # 2500 real classes per core
NBLK = 20                    # class blocks of 128 (last has 60 pad)
CSH = NBLK * 128             # 2560
NPAIR = NBLK // 2            # 10 block pairs per row-tile
NRT = 8                      # row tiles of 512
RTW = B // NRT               # 512 rows per tile

AM_MARGIN = 0.3
AM_SCALE = 30.0
INTRA_MARGIN = 0.5
LAMBDA_INTRA = 0.1
OFF = 6.0                    # logsumexp offset (e^(z-OFF) stays in e5m2 range)
QS = 16.0                    # fp8 quantization scale on each operand
ZSCALE = AM_SCALE / (QS * QS)   # psum -> s*cos

# e5m2-space Schraudolph: u8 = round(z*A8 + B8) is e5m2 bits of ~exp(z);
# c8 calibrated so the piecewise-linear interp + RNE is sum-unbiased.
A8 = 4.0 / np.log(2.0)
C8 = 0.2248
SCHM = float(A8 * ZSCALE)
SCHB = float(4 * 15 - C8 - A8 * OFF)
# ACT fp8-RNE sum bias correction (ratio 0.997158 measured on the input dist)
ACT_BIAS = float(-OFF + 0.002846)

F32 = mybir.dt.float32
U8 = mybir.dt.uint8
F8E4 = mybir.dt.float8e4
F8E5 = mybir.dt.float8e5
AF = mybir.ActivationFunctionType
ALU = mybir.AluOpType
PM = mybir.MatmulPerfMode

# pair -> engine schedule, balanced by modeled cost (ACT pair 1039 ns,
# DVE pair 1192 ns), ACT seeded with its 1283 ns activation-table load.
ACT_PAIR_NS = 1039.0
DVE_PAIR_NS = 1192.0


def _make_schedule():
    # strict alternation: same per-engine totals as the greedy split but no
    # A-A adjacencies (those trigger pipeline hiccups in the 3-slot ring)
    return ["D" if i % 2 == 0 else "A" for i in range(NRT * NPAIR)]


SCHEDULE = _make_schedule()


def build_program():
    nc = bacc.Bacc("TRN2", target_bir_lowering=False)

    etq_d = nc.dram_tensor("etq", [128, 2, B], F8E4, kind="ExternalInput")
    wtq_d = nc.dram_tensor("wtq", [128, 2, CSH], F8E4, kind="ExternalInput")
    outs_d = nc.dram_tensor("out_s", [128, NRT * 4], F32, kind="ExternalOutput")

    with tile.TileContext(nc) as tc, ExitStack() as ctx:
        big = ctx.enter_context(tc.tile_pool(name="big", bufs=1))
        scr = ctx.enter_context(tc.tile_pool(name="scr", bufs=1))
        psum = ctx.enter_context(tc.tile_pool(name="psum", bufs=1, space="PSUM"))

        ETQ = big.tile([128, 2, B], F8E4)
        WTQ = big.tile([128, 2, CSH], F8E4)
        negoff = big.tile([128, 1], F32)
        ones8 = big.tile([128, 2, 1], F8E5)
        nc.vector.memset(negoff, ACT_BIAS)
        nc.gpsimd.memset(ones8, 1.0)

        # force the activation-table load during the DMA wait
        junkA = scr.tile([128, 1], F32, tag="ja")
        nc.scalar.activation(out=junkA, in_=negoff, func=AF.Exp,
                             scale=1.0, bias=negoff[:, 0:1])

        # DMAs: the first pairs need wtq blocks 0.. + etq row-tile 0; spread
        # the critical pieces across the SP / ACT hwdge queues + Pool swdge.
        nc.sync.dma_start(out=WTQ[:, :, 0:256], in_=wtq_d[:][:, :, 0:256])
        nc.scalar.dma_start(out=ETQ[:, :, 0:512], in_=etq_d[:][:, :, 0:512])
        nc.gpsimd.dma_start(out=WTQ[:, :, 1024:2560], in_=wtq_d[:][:, :, 1024:2560])
        nc.sync.dma_start(out=WTQ[:, :, 256:1024], in_=wtq_d[:][:, :, 256:1024])
        nc.scalar.dma_start(out=ETQ[:, :, 2048:4096], in_=etq_d[:][:, :, 2048:4096])
        nc.sync.dma_start(out=ETQ[:, :, 512:2048], in_=etq_d[:][:, :, 512:2048])

        psS = psum.tile([128, NRT * 4], F32, tag="s", bufs=1)

        # main loop: 8 row-tiles x 10 class-block pairs. Sums for row-tile rt
        # are emitted after the mains of rt+1 so PE dispatch never waits on a
        # transform while psZ slots for upcoming pairs still need filling.
        from collections import deque
        pending = deque()
        si = 0

        def emit_sums():
            csch, crt, cp = pending.popleft()
            for R in range(4):
                nc.tensor.matmul(
                    psS[:, crt * 4 + R:crt * 4 + R + 1],
                    lhsT=csch[:, :, R * 128:(R + 1) * 128],
                    rhs=ones8, start=(cp == 0), stop=(cp == NPAIR - 1),
                    perf_mode=PM.DoubleRow)

        for rt in range(NRT):
            erows = ETQ[:, :, rt * RTW:(rt + 1) * RTW]
            for p in range(NPAIR):
                if len(pending) >= 10:
                    emit_sums()
                psZ = psum.tile([128, 1024], F32, tag="z", bufs=3)
                nc.tensor.matmul(psZ[:, 0:512],
                                 lhsT=WTQ[:, :, (2 * p) * 128:(2 * p + 1) * 128],
                                 rhs=erows, start=True, stop=True,
                                 perf_mode=PM.DoubleRow)
                nc.tensor.matmul(psZ[:, 512:1024],
                                 lhsT=WTQ[:, :, (2 * p + 1) * 128:(2 * p + 2) * 128],
                                 rhs=erows, start=True, stop=True,
                                 perf_mode=PM.DoubleRow)
                if SCHEDULE[si] == "A":
                    sch = scr.tile([128, 2, 512], F8E5, tag="sa", bufs=30)
                    nc.scalar.activation(out=sch, in_=psZ, func=AF.Exp,
                                         scale=ZSCALE, bias=negoff[:, 0:1])
                else:
                    schu = scr.tile([128, 2, 512], U8, tag="sd", bufs=30)
                    nc.vector.tensor_scalar(out=schu, in0=psZ,
                                            scalar1=SCHM, scalar2=SCHB,
                                            op0=ALU.mult, op1=ALU.add)
                    sch = schu.bitcast(F8E5)
                si += 1
                pending.append((sch, rt, p))

        while pending:
            emit_sums()

        ssb = big.tile([128, NRT * 4], F32)
        nc.vector.tensor_copy(out=ssb, in_=psS)
        nc.sync.dma_start(out=outs_d[:], in_=ssb)

    nc.finalize()
    return nc


def kernel(embeddings, labels, weight):
    e = np.ascontiguousarray(embeddings, dtype=np.float32)
    lab = np.asarray(labels).astype(np.int64)
    w = np.ascontiguousarray(weight, dtype=np.float32)
    assert e.shape == (B, D) and w.shape == (C, D) and lab.shape == (B,)

    En = (e / np.linalg.norm(e, axis=1, keepdims=True)).astype(np.float32)
    Wn = (w / np.linalg.norm(w, axis=1, keepdims=True)).astype(np.float32)
    Eq = (QS * En).astype(ml_dtypes.float8_e4m3fn)
    etq = np.ascontiguousarray(
        Eq.T.reshape(2, 128, B).transpose(1, 0, 2))          # [128, 2, B]

    in_maps = []
    for k in range(NCORES):
        wsh = np.zeros((CSH, D), np.float32)
        wsh[:CREAL] = Wn[k * CREAL:(k + 1) * CREAL]
        Wq = (QS * wsh).astype(ml_dtypes.float8_e4m3fn)
        wtq = np.ascontiguousarray(Wq.T.reshape(2, 128, CSH).transpose(1, 0, 2))
        in_maps.append({"etq": etq, "wtq": wtq})

    nc = build_program()
    res = run_bass_kernel_spmd(nc, in_maps, core_ids=list(range(NCORES)))
    global _last_results
    _last_results = res

    # ---------------- host combine (float64) -----------------------------
    # out_s[:, rt*4 + R] column = rows rt*512 + R*128 + [0, 128)
    S = np.zeros(B, np.float64)
    for k in range(NCORES):
        o = res.results[k]["out_s"].astype(np.float64)       # [128, 32]
        S += o.T.reshape(B)                                  # rt,R,p -> row

    # padded classes (60 per core, z=0) all went through whichever engine
    # owned pair 9; both engines map z=0 to the same e5m2 value, computed
    # here exactly as the device does.
    pad_bits = np.uint8(np.rint(SCHB))
    y_pad_dve = float(np.asarray(pad_bits.view(ml_dtypes.float8_e5m2), np.float64))
    y_pad_act = float(np.asarray(
        np.float32(np.exp(ACT_BIAS)).astype(ml_dtypes.float8_e5m2), np.float64))
    n_pad = CSH - CREAL
    # pair 9 (which holds the pad block) engine varies by row-tile; every
    # core contributes n_pad * y_pad(engine) to each row of that row-tile.
    for rt in range(NRT):
        eng = SCHEDULE[rt * NPAIR + NPAIR - 1]
        y_pad = y_pad_act if eng == "A" else y_pad_dve
        S[rt * RTW:(rt + 1) * RTW] -= NCORES * n_pad * y_pad

    cl = np.einsum("bd,bd->b", En.astype(np.float64),
                   Wn.astype(np.float64)[lab])               # exact label cos
    s, m = float(AM_SCALE), float(AM_MARGIN)
    S_adj = S - np.exp(s * cl - OFF) + np.exp(s * (cl - m) - OFF)
    am_i = (np.log(S_adj) + OFF) - s * (cl - m)
    am = am_i.mean()

    members = np.argsort(lab, kind="stable").reshape(G, NSAMP)
    Eg = En.astype(np.float64)[members]                      # [G, 8, D]
    gs = Eg.sum(axis=1)                                      # [G, D]
    npairs = NSAMP * (NSAMP - 1) / 2.0
    dsum = npairs - ((gs * gs).sum(axis=1) - NSAMP) / 2.0
    per_group = np.maximum(dsum / npairs - INTRA_MARGIN, 0.0)
    intra = per_group.mean()

    total = am + LAMBDA_INTRA * intra
    return (np.float32(total), np.float32(am), np.float32(intra))


# revision 32
# speedup vs baseline: 1.1257x; 1.0144x over previous
"""
AM-Softmax + intra-class loss kernel for Trainium2, 8 NeuronCores.

Strategy (class-sharded, transposed-Z fp8 pipeline):
  * Classes C=20000 sharded 2500/core (padded to 20 blocks of 128). Host
    normalizes E and W rows, scales by 16, casts to fp8-e4m3 (the per-row
    scale fp8 needs anyway); the AM scale and quant factors ride the exp
    transforms' constants.
  * Z is computed TRANSPOSED: per (class-block pair, 512-row tile) two fp8
    DoubleRow matmuls emit a [128 classes, 1024] PSUM tile (lhsT=W-block,
    rhs=E-tile); pairs stream through 3 double-buffered psum slots.
  * exp: pair tiles strictly alternate between ACT (native Exp -> fp8-e5m2
    out) and DVE (Schraudolph: u8 = round(a*z+b) is the e5m2 bit pattern of
    ~exp(z-OFF); the saturating f32->u8 conversion clamps underflow to +0).
  * The softmax row-sum is a PE ones-matmul over partitions: per pair and
    128-row slice, out[row, 1] accumulates into a [128, 128] psum tile over
    the 10 pairs of each row-tile (output free size 1 -> negligible PE
    time). No DVE fold passes, no ACT accumulator reads. Sum columns sit 16
    bytes apart because a chain-opening matmul (start=True) zeroes its
    whole 16-byte-aligned psum block on hardware.
  * PE pstate warmup: junk matmuls during the input-DMA wait so real mains
    run at full clock; input DMA is split across the SP/ACT/Pool queues by
    urgency (first W blocks + first E row-tile land first).
  * One [128, 128] copy + DMA (split 112/16 so most ships early) returns
    all 4096 partial sums per core.
  * Label-cos dots and the intra-class term are O(B*D) epilogue work and
    run on host in float64 (the host already does the O((B+C)*D)
    normalize / quantize prologue).
"""

import numpy as np
import ml_dtypes

import concourse.bacc as bacc
import concourse.tile as tile
from concourse import mybir
from concourse.bass_utils import run_bass_kernel_spmd
from contextlib import ExitStack

B = 4096
D = 256
C = 20000
G = 512
NSAMP = 8
NCORES = 8
CREAL = C // NCORES          # 2500 real classes per core
NBLK = 20                    # class blocks of 128 (last has 60 pad)
CSH = NBLK * 128             # 2560
NPAIR = NBLK // 2            # 10 block pairs per row-tile
NRT = 8                      # row tiles of 512
RTW = B // NRT               # 512 rows per tile

AM_MARGIN = 0.3
AM_SCALE = 30.0
INTRA_MARGIN = 0.5
LAMBDA_INTRA = 0.1
OFF = 6.0                    # logsumexp offset (e^(z-OFF) stays in e5m2 range)
QS = 16.0                    # fp8 quantization scale on each operand
ZSCALE = AM_SCALE / (QS * QS)   # psum -> s*cos

# e5m2-space Schraudolph: u8 = round(z*A8 + B8) is e5m2 bits of ~exp(z);
# c8 calibrated so the piecewise-linear interp + RNE is sum-unbiased.
A8 = 4.0 / np.log(2.0)
C8 = 0.2248
SCHM = float(A8 * ZSCALE)
SCHB = float(4 * 15 - C8 - A8 * OFF)
# ACT fp8-RNE sum bias correction (ratio 0.997158 measured on the input dist)
ACT_BIAS = float(-OFF + 0.002846)

F32 = mybir.dt.float32
U8 = mybir.dt.uint8
F8E4 = mybir.dt.float8e4
F8E5 = mybir.dt.float8e5
AF = mybir.ActivationFunctionType
ALU = mybir.AluOpType
PM = mybir.MatmulPerfMode

# pair -> engine schedule, balanced by modeled cost (ACT pair 1039 ns,
# DVE pair 1192 ns), ACT seeded with its 1283 ns activation-table load.
ACT_PAIR_NS = 1039.0
DVE_PAIR_NS = 1192.0


def _make_schedule():
    # strict alternation: same per-engine totals as the greedy split but no
    # A-A adjacencies (those trigger pipeline hiccups in the 3-slot ring).
    # The final pair goes to ACT, which otherwise finishes ~1.5us early.
    s = ["A" if i % 2 == 0 else "D" for i in range(NRT * NPAIR)]
    s[77] = "A"
    return s


SCHEDULE = _make_schedule()


def build_program():
    nc = bacc.Bacc("TRN2", target_bir_lowering=False)

    etq_d = nc.dram_tensor("etq", [128, 2, B], F8E4, kind="ExternalInput")
    wtq_d = nc.dram_tensor("wtq", [128, 2, CSH], F8E4, kind="ExternalInput")
    outs_d = nc.dram_tensor("out_s", [128, NRT * 16], F32, kind="ExternalOutput")

    with tile.TileContext(nc) as tc, ExitStack() as ctx:
        big = ctx.enter_context(tc.tile_pool(name="big", bufs=1))
        scr = ctx.enter_context(tc.tile_pool(name="scr", bufs=1))
        psum = ctx.enter_context(tc.tile_pool(name="psum", bufs=1, space="PSUM"))

        ETQ = big.tile([128, 2, B], F8E4)
        WTQ = big.tile([128, 2, CSH], F8E4)
        negoff = big.tile([128, 1], F32)
        # wide ones: [:, :, 0:1] feeds the sum quads; the full tile is the
        # rhs/lhsT of the PE-warmup junk matmuls
        ones8 = big.tile([128, 2, 512], F8E5)
        nc.vector.memset(negoff, ACT_BIAS)
        nc.gpsimd.memset(ones8, 1.0)

        # DMAs: the first pairs need wtq blocks 0.. + etq row-tile 0; spread
        # the critical pieces across the SP / ACT hwdge queues + Pool swdge.
        nc.sync.dma_start(out=WTQ[:, :, 0:256], in_=wtq_d[:][:, :, 0:256])
        nc.scalar.dma_start(out=ETQ[:, :, 0:512], in_=etq_d[:][:, :, 0:512])
        nc.sync.dma_start(out=WTQ[:, :, 256:768], in_=wtq_d[:][:, :, 256:768])
        nc.sync.dma_start(out=WTQ[:, :, 768:1536], in_=wtq_d[:][:, :, 768:1536])
        nc.sync.dma_start(out=WTQ[:, :, 1536:2560], in_=wtq_d[:][:, :, 1536:2560])
        # bulk ETQ rides the idle Pool swdge queue so its long transfers
        # never cut ahead of the urgent WTQ pieces on the shared dma lane
        nc.gpsimd.dma_start(out=ETQ[:, :, 512:1024], in_=etq_d[:][:, :, 512:1024])
        nc.sync.dma_start(out=ETQ[:, :, 1024:4096], in_=etq_d[:][:, :, 1024:4096])

        # force the activation-table load during the DMA wait (after the ACT
        # queue's first DMA so it isn't delayed behind the table load)
        junkA = scr.tile([128, 1], F32, tag="ja")
        nc.scalar.activation(out=junkA, in_=negoff, func=AF.Exp,
                             scale=1.0, bias=negoff[:, 0:1])

        # sum columns live 16 bytes apart: a chain-opening matmul
        # (start=True) zeroes its whole 16-byte-aligned psum block on HW,
        # so each live column gets its own block.
        psS = psum.tile([128, NRT * 16], F32, tag="s", bufs=1)

        # PE pstate warmup: ~8 junk matmuls into the spare bank during the
        # input-DMA wait so the first real mains run at full clock.
        psJ = psum.tile([128, 512], F32, tag="warm", bufs=1)
        for _ in range(7):
            nc.tensor.matmul(psJ, lhsT=ones8[:, :, 0:128], rhs=ones8,
                             start=True, stop=True, perf_mode=PM.DoubleRow)

        # main loop: 8 row-tiles x 10 class-block pairs. Sums for row-tile rt
        # are emitted after the mains of rt+1 so PE dispatch never waits on a
        # transform while psZ slots for upcoming pairs still need filling.
        from collections import deque
        pending = deque()
        si = 0

        def emit_sums():
            csch, crt, cp = pending.popleft()
            for R in range(4):
                nc.tensor.matmul(
                    psS[:, (crt * 4 + R) * 4:(crt * 4 + R) * 4 + 1],
                    lhsT=csch[:, :, R * 128:(R + 1) * 128],
                    rhs=ones8[:, :, 0:1], start=(cp == 0), stop=(cp == NPAIR - 1),
                    perf_mode=PM.DoubleRow)

        for rt in range(NRT):
            erows = ETQ[:, :, rt * RTW:(rt + 1) * RTW]
            for p in range(NPAIR):
                if len(pending) >= 6:
                    emit_sums()
                psZ = psum.tile([128, 1024], F32, tag="z", bufs=3)
                nc.tensor.matmul(psZ[:, 0:512],
                                 lhsT=WTQ[:, :, (2 * p) * 128:(2 * p + 1) * 128],
                                 rhs=erows, start=True, stop=True,
                                 perf_mode=PM.DoubleRow)
                nc.tensor.matmul(psZ[:, 512:1024],
                                 lhsT=WTQ[:, :, (2 * p + 1) * 128:(2 * p + 2) * 128],
                                 rhs=erows, start=True, stop=True,
                                 perf_mode=PM.DoubleRow)
                if SCHEDULE[si] == "A":
                    sch = scr.tile([128, 2, 512], F8E5, tag="sa", bufs=30)
                    nc.scalar.activation(out=sch, in_=psZ, func=AF.Exp,
                                         scale=ZSCALE, bias=negoff[:, 0:1])
                else:
                    schu = scr.tile([128, 2, 512], U8, tag="sd", bufs=30)
                    nc.vector.tensor_scalar(out=schu, in0=psZ,
                                            scalar1=SCHM, scalar2=SCHB,
                                            op0=ALU.mult, op1=ALU.add)
                    sch = schu.bitcast(F8E5)
                si += 1
                pending.append((sch, rt, p))

        while len(pending) > NPAIR:
            emit_sums()

        # columns 0:28 (row-tiles 0-6) are final; ship them while the last
        # row-tile's sums drain
        ssb = big.tile([128, NRT * 16], F32)
        nc.vector.tensor_copy(out=ssb[:, 0:112], in_=psS[:, 0:112])
        nc.scalar.dma_start(out=outs_d[:][:, 0:112], in_=ssb[:, 0:112])

        while pending:
            emit_sums()

        nc.vector.tensor_copy(out=ssb[:, 112:128], in_=psS[:, 112:128])
        nc.sync.dma_start(out=outs_d[:][:, 112:128], in_=ssb[:, 112:128])

    nc.finalize()
    return nc


def kernel(embeddings, labels, weight):
    e = np.ascontiguousarray(embeddings, dtype=np.float32)
    lab = np.asarray(labels).astype(np.int64)
    w = np.ascontiguousarray(weight, dtype=np.float32)
    assert e.shape == (B, D) and w.shape == (C, D) and lab.shape == (B,)

    En = (e / np.linalg.norm(e, axis=1, keepdims=True)).astype(np.float32)
    Wn = (w / np.linalg.norm(w, axis=1, keepdims=True)).astype(np.float32)
    Eq = (QS * En).astype(ml_dtypes.float8_e4m3fn)
    etq = np.ascontiguousarray(
        Eq.T.reshape(2, 128, B).transpose(1, 0, 2))          # [128, 2, B]

    in_maps = []
    for k in range(NCORES):
        wsh = np.zeros((CSH, D), np.float32)
        wsh[:CREAL] = Wn[k * CREAL:(k + 1) * CREAL]
        Wq = (QS * wsh).astype(ml_dtypes.float8_e4m3fn)
        wtq = np.ascontiguousarray(Wq.T.reshape(2, 128, CSH).transpose(1, 0, 2))
        in_maps.append({"etq": etq, "wtq": wtq})

    nc = build_program()
    res = run_bass_kernel_spmd(nc, in_maps, core_ids=list(range(NCORES)))
    global _last_results
    _last_results = res

    # ---------------- host combine (float64) -----------------------------
    # out_s[:, (rt*4 + R)*4] column = rows rt*512 + R*128 + [0, 128)
    S = np.zeros(B, np.float64)
    for k in range(NCORES):
        o = res.results[k]["out_s"].astype(np.float64)       # [128, 128]
        S += o[:, ::4].T.reshape(B)                          # rt,R,p -> row

    # padded classes (60 per core, z=0) all went through whichever engine
    # owned pair 9; both engines map z=0 to the same e5m2 value, computed
    # here exactly as the device does.
    pad_bits = np.uint8(np.rint(SCHB))
    y_pad_dve = float(np.asarray(pad_bits.view(ml_dtypes.float8_e5m2), np.float64))
    y_pad_act = float(np.asarray(
        np.float32(np.exp(ACT_BIAS)).astype(ml_dtypes.float8_e5m2), np.float64))
    n_pad = CSH - CREAL
    # pair 9 (which holds the pad block) engine varies by row-tile; every
    # core contributes n_pad * y_pad(engine) to each row of that row-tile.
    for rt in range(NRT):
        eng = SCHEDULE[rt * NPAIR + NPAIR - 1]
        y_pad = y_pad_act if eng == "A" else y_pad_dve
        S[rt * RTW:(rt + 1) * RTW] -= NCORES * n_pad * y_pad

    cl = np.einsum("bd,bd->b", En.astype(np.float64),
                   Wn.astype(np.float64)[lab])               # exact label cos
    s, m = float(AM_SCALE), float(AM_MARGIN)
    S_adj = S - np.exp(s * cl - OFF) + np.exp(s * (cl - m) - OFF)
    am_i = (np.log(S_adj) + OFF) - s * (cl - m)
    am = am_i.mean()

    members = np.argsort(lab, kind="stable").reshape(G, NSAMP)
    Eg = En.astype(np.float64)[members]                      # [G, 8, D]
    gs = Eg.sum(axis=1)                                      # [G, D]
    npairs = NSAMP * (NSAMP - 1) / 2.0
    dsum = npairs - ((gs * gs).sum(axis=1) - NSAMP) / 2.0
    per_group = np.maximum(dsum / npairs - INTRA_MARGIN, 0.0)
    intra = per_group.mean()

    total = am + LAMBDA_INTRA * intra
    return (np.float32(total), np.float32(am), np.float32(intra))
